# revision 33
# baseline (speedup 1.0000x reference)
"""Trainium2 Bass kernel for nn_DetLoss_3762391351632.

Data-parallel over batch: 8 images -> 8 NeuronCores, one image per core.
Each core emits 5 partial scalars; host assembles & averages (the
"all-reduce mean" of the sharding hint, done on 8 floats).

Per-core algorithm (validated vs the reference in numpy, rel err ~3e-6):
  grid layout [128 partitions x 800]; partition p owns anchors
  [800p, 800p+800) for p<125; 3 pad partitions are masked out.
  - IoU in u-space: u = inter/(anchor_area+box_area), monotone in IoU;
    IoU thresholds 0.5/0.4 become u thresholds 1/3, 2/7.
  - per-anchor argmax over the 32 annotations and per-annotation argmax
    over all anchors via bit-packed quantized max (index in the low
    mantissa bits; bit30 forced on so packed values stay normal floats).
  - the reference's sequential low-quality-match scan == last-writer-wins
    scatter of <=32 values (dedup on a 32x32 tile + gpsimd local_scatter).
  - focal cls loss: sum over not-ignored rows of (1-a)*c^2*(-ln(1-c))
    via a PE trace trick: Square and Ln(1-c) as bf16 ACT outputs, their
    Frobenius inner product accumulated as 250 128x128 matmuls into one
    PSUM tile, diagonal extracted with a mask; plus exact per-anchor
    corrections at the assigned class.
  - annotation-field and class-column gathers are 32/40-way masked
    copy_predicated selects (no per-partition indexed gather exists).
"""
import math
import sys

sys.path.insert(0, "/opt/trn_rl_repo")

import numpy as np

import concourse.bass as bass
import concourse.bacc as bacc
import concourse.mybir as mybir
from concourse import bass_isa
from concourse.tile import TileContext
from concourse import dve_ops as _dvo
from concourse import dve_spec as _dvs
from concourse.dve_uop import DveOpSpec as _DveOpSpec


def _reg_dve_op(name, body, reference):
    """Register a custom fused DVE op at runtime (row in [1,0x20))."""
    for o in _dvo.OPS:
        if o.name == name:
            return o
    spec = _dvs.Spec(body=body, reference=reference)
    row = max(_dvo._SUB_OPCODE_FOR_NAME.values()) + 1
    assert row < 0x20
    _dvo._SUB_OPCODE_FOR_NAME[name] = row
    shas = {}
    for ver in ("v3", "v4"):
        try:
            uops = _dvs.lower(spec, ver=ver)
            shas[ver] = _DveOpSpec(name=name, opcode=row, uops=uops,
                                   rd1_en=_dvs._has_src1(spec)).sha(ver)
        except Exception:
            pass
    assert shas, f"custom DVE op {name} failed to lower"
    op = _dvo.DveOp(name, spec, False, shas)
    _dvo.OPS.append(op)
    _dvo.CUSTOM_DVE_SPECS[name] = spec
    return op


def _iwc_ref(in0, in1, s0, s1, imm2):
    return np.maximum(np.minimum(in0, s0) - np.maximum(in1, s1), 0.0)


def _mp_ref(in0, in1, s0, s1, imm2):
    p = (np.asarray(in0, np.float32) * np.asarray(in1, np.float32)).astype(np.float32)
    m = np.array(s0, np.float32).view(np.uint32)
    c = np.array(s1, np.float32).view(np.uint32)
    return ((p.view(np.uint32) & m) | c).view(np.float32)


# iwc = relu(min(x2, bx2) - max(x1, bx1)) : one pass instead of act+ts+tt+act
IWC_OP = _reg_dve_op(
    "ANT_IWC_CLIP",
    _dvs.relu(_dvs.minn(_dvs.Src0, _dvs.C0) - _dvs.maxx(_dvs.Src1, _dvs.C1)),
    _iwc_ref)
# jpk = ((inter * r) & mask) | code : fuses u-mult + quantize-pack
MULPACK_OP = _reg_dve_op(
    "ANT_MULPACK",
    _dvs.Bin(_dvs.AluOp.BITWISE_OR,
             _dvs.Bin(_dvs.AluOp.BITWISE_AND, _dvs.Src0 * _dvs.Src1, _dvs.C0),
             _dvs.C1),
    _mp_ref)

f32 = np.float32
dt = mybir.dt
ALU = mybir.AluOpType
ACTF = mybir.ActivationFunctionType
AX = mybir.AxisListType

A, M, C = 100000, 32, 40
P, PA, G = 128, 125, 800
NCHUNK = 20
GC = G // NCHUNK          # 80 anchors / partition / chunk
CHF = GC * C              # 3200 elems / partition / chunk
ALPHA = f32(0.25)
HI = float(f32(1.0 - 1e-4))
HI32 = f32(1.0 - 1e-4)
LO = float(f32(1e-4))
REG_W = f32(5.0)
T13 = float(f32(1.0 / 3.0))
T27 = float(f32(2.0 / 7.0))
BIT30 = 0x40000000
N_OUT = 8


def host_constants():
    g = np.arange(G, dtype=np.uint32)
    gcode = np.broadcast_to(((1023 - g) | BIT30)[None, :], (P, G)).copy()
    pio128 = np.broadcast_to(np.arange(P, dtype=f32)[None, :], (M, P)).copy()

    gio800 = np.broadcast_to(np.arange(G, dtype=f32)[None, :], (M, G)).copy()
    onesb = np.ones((1, P), dtype=f32)
    onesc = np.ones((P, 1), dtype=f32)
    jp1c = np.arange(1, M + 1, dtype=f32)[:, None]
    lt = (np.arange(M)[:, None] > np.arange(M)[None, :]).astype(f32)
    ident = np.eye(P, dtype=f32)
    vmask = (np.arange(P * G).reshape(P, G) < A).astype(f32)
    iota32 = (np.arange(P) % 32).astype(f32)[:, None]
    qsel = np.zeros((4, P), dtype=f32)
    for q in range(4):
        qsel[q, 32 * q:32 * q + 32] = 1.0
    pkc = np.zeros((P, 33), dtype=np.uint32)
    pkc[:, 0] = 0xFFFFFFE0
    for j in range(M):
        pkc[:, 1 + j] = (31 - j) | BIT30
    pkc = pkc.view(f32)
    return {"gcode": gcode, "pio128": pio128, "gio800": gio800, "onesb": onesb,
            "onesc": onesc, "jp1c": jp1c, "ltmask": lt, "ident": ident,
            "vmask": vmask, "iota32": iota32, "qsel": qsel, "pkconst": pkc}


def build_bass(debug=False):
    nc = bacc.Bacc()
    cls_d = nc.declare_dram_parameter("classification", [P * G, C], dt.float32, isOutput=False)
    reg_d = nc.declare_dram_parameter("regression", [P * G, 4], dt.float32, isOutput=False)
    anc_d = nc.declare_dram_parameter("anchors", [P * G, 4], dt.float32, isOutput=False)
    ann_d = nc.declare_dram_parameter("annotation", [M, 5], dt.float32, isOutput=False)
    gcode_d = nc.declare_dram_parameter("gcode", [P, G], dt.uint32, isOutput=False)
    pio128_d = nc.declare_dram_parameter("pio128", [M, P], dt.float32, isOutput=False)
    gio800_d = nc.declare_dram_parameter("gio800", [M, G], dt.float32, isOutput=False)
    onesb_d = nc.declare_dram_parameter("onesb", [1, P], dt.float32, isOutput=False)
    onesc_d = nc.declare_dram_parameter("onesc", [P, 1], dt.float32, isOutput=False)
    jp1c_d = nc.declare_dram_parameter("jp1c", [M, 1], dt.float32, isOutput=False)
    lt_d = nc.declare_dram_parameter("ltmask", [M, M], dt.float32, isOutput=False)
    ident_d = nc.declare_dram_parameter("ident", [P, P], dt.float32, isOutput=False)
    vmask_d = nc.declare_dram_parameter("vmask", [P, G], dt.float32, isOutput=False)
    iota32_d = nc.declare_dram_parameter("iota32", [P, 1], dt.float32, isOutput=False)
    qsel_d = nc.declare_dram_parameter("qsel", [4, P], dt.float32, isOutput=False)
    pkc_d = nc.declare_dram_parameter("pkconst", [P, 33], dt.float32, isOutput=False)
    out_d = nc.declare_dram_parameter("out", [N_OUT], dt.float32, isOutput=True)
    dbg = {}
    if debug:
        for nm, shape, dty in [
            ("dbg_umaxq", [P, G], dt.float32), ("dbg_w0", [P, G], dt.float32),
            ("dbg_pos", [P, G], dt.float32), ("dbg_jeff", [P, G], dt.float32),
            ("dbg_csel", [P, G], dt.float32), ("dbg_colpk", [P, M], dt.uint32),
            ("dbg_rowpk", [P, G], dt.uint32), ("dbg_ovc", [P, G], dt.float32),
            ("dbg_u5", [P, G], dt.float32), ("dbg_clsg", [P, G], dt.float32),
            ("dbg_rsum", [P, G], dt.float32),
        ]:
            dbg[nm] = nc.declare_dram_parameter(nm, shape, dty, isOutput=True)

    v = nc.vector
    s = nc.scalar
    gp = nc.gpsimd
    te = nc.tensor

    with TileContext(nc) as tc:
        with (
            tc.tile_pool(name="const", bufs=1) as constp,
            tc.tile_pool(name="planes", bufs=1) as pl,
            tc.tile_pool(name="tmp", bufs=1) as tp,
            tc.tile_pool(name="chunks", bufs=2) as chp,
            tc.tile_pool(name="small", bufs=1) as sm,
            tc.tile_pool(name="smtmp", bufs=2) as st,
            tc.tile_pool(name="psum", bufs=2, space="PSUM") as pp,
        ):
            # ---------- constants ----------
            gcode = constp.tile([P, G], dt.uint32, name="gcode", tag="gcode")
            nc.sync.dma_start(gcode[:], gcode_d[:, :])
            pio128 = constp.tile([M, P], dt.float32, name="pio128", tag="pio128")
            nc.sync.dma_start(pio128[:], pio128_d[:, :])
            gio800 = constp.tile([M, G], dt.float32, name="gio800", tag="gio800")
            nc.sync.dma_start(gio800[:], gio800_d[:, :])
            onesb = constp.tile([1, P], dt.float32, name="onesb", tag="onesb")
            nc.sync.dma_start(onesb[:], onesb_d[:, :])
            onesc = constp.tile([P, 1], dt.float32, name="onesc", tag="onesc")
            nc.sync.dma_start(onesc[:], onesc_d[:, :])
            jp1c = constp.tile([M, 1], dt.float32, name="jp1c", tag="jp1c")
            nc.sync.dma_start(jp1c[:], jp1c_d[:, :])
            ltm = constp.tile([M, M], dt.float32, name="ltm", tag="ltm")
            nc.sync.dma_start(ltm[:], lt_d[:, :])
            ident = constp.tile([P, P], dt.float32, name="ident", tag="ident")
            nc.sync.dma_start(ident[:], ident_d[:, :])
            vmask = constp.tile([P, G], dt.float32, name="vmask", tag="vmask")
            nc.sync.dma_start(vmask[:], vmask_d[:, :])
            iota32 = constp.tile([P, 1], dt.float32, name="iota32", tag="iota32")
            nc.sync.dma_start(iota32[:], iota32_d[:, :])
            qsel = constp.tile([4, P], dt.float32, name="qsel", tag="qsel")
            nc.sync.dma_start(qsel[:], qsel_d[:, :])
            pkc = constp.tile([P, 33], dt.float32, name="pkc", tag="pkc")
            nc.sync.dma_start(pkc[:], pkc_d[:, :])
            biasc = constp.tile([P, 2], dt.float32, name="biasc", tag="biasc")
            v.memset(biasc[:, 0:1], float(f32(math.pi / 2)))
            v.memset(biasc[:, 1:2], -1.0)


            def ts_bits(out_ap, in0_ap, s1, op0, s2=None, op1=None, eng=None):
                e = eng if eng is not None else v
                ins = [e.lower_ap(in0_ap),
                       mybir.ImmediateValue(dtype=dt.uint32, value=int(s1))]
                if s2 is not None:
                    ins.append(mybir.ImmediateValue(dtype=dt.uint32, value=int(s2)))
                e.add_instruction(mybir.InstTensorScalarPtr(
                    name=nc.get_next_instruction_name(),
                    op0=op0, op1=(op1 if op1 is not None else ALU.bypass),
                    ins=ins, outs=[e.lower_ap(out_ap)]))

            def stt_bits(out_ap, in0_ap, s1, in1_ap, op0, op1, eng=None):
                e = eng if eng is not None else v
                ins = [e.lower_ap(in0_ap),
                       mybir.ImmediateValue(dtype=dt.uint32, value=int(s1)),
                       e.lower_ap(in1_ap)]
                e.add_instruction(mybir.InstTensorScalarPtr(
                    name=nc.get_next_instruction_name(),
                    is_scalar_tensor_tensor=True,
                    op0=op0, op1=op1,
                    ins=ins, outs=[e.lower_ap(out_ap)]))

            # ---------- anchors ----------
            anc = pl.tile([P, 4 * G], dt.float32, name="anc", tag="anc")
            nc.sync.dma_start(anc[:, :], anc_d.rearrange("(p g) c -> p (g c)", p=P))
            x1 = anc[:, 0:4 * G:4]
            y1 = anc[:, 1:4 * G:4]
            x2 = anc[:, 2:4 * G:4]
            y2 = anc[:, 3:4 * G:4]

            aa = pl.tile([P, G], dt.float32, name="aa", tag="aa")
            aw = pl.tile([P, G], dt.float32, name="aw", tag="aw")
            ah = pl.tile([P, G], dt.float32, name="ah", tag="ah")
            v.tensor_tensor(aw[:], x2, x1, op=ALU.subtract)
            v.tensor_tensor(ah[:], y2, y1, op=ALU.subtract)
            v.tensor_tensor(aa[:], aw[:], ah[:], op=ALU.mult)

            # ---------- annotation prep (rows on partitions 0..4) ----------
            annT = sm.tile([1, 5 * M], dt.float32, name="annT", tag="annT")
            with nc.allow_non_contiguous_dma(reason="tiny 32x5 transposed load"):
                nc.sync.dma_start(annT[:].rearrange("o (f m) -> o f m", m=M), ann_d.rearrange("m f -> f m")[None, :, :])
            cxr, cyr, thr, lnr, clsr = (annT[:, i * M:(i + 1) * M] for i in range(5))

            valid_r = sm.tile([1, M], dt.float32, name="valid", tag="valid")
            v.tensor_scalar(valid_r[:], clsr, -1.0, None, op0=ALU.not_equal)
            wk = lambda tag: st.tile([1, M], dt.float32, name=tag, tag=tag)
            cosv, sinv, dxv, dyv = wk("cosv"), wk("sinv"), wk("dxv"), wk("dyv")
            s.activation(cosv[:], thr, ACTF.Sin, bias=biasc[0:1, 0:1], scale=-1.0)
            s.activation(sinv[:], thr, ACTF.Sin)
            t0 = wk("t0")
            v.tensor_tensor(t0[:], lnr, cosv[:], op=ALU.mult)
            s.activation(dxv[:], t0[:], ACTF.Abs, scale=0.5)
            v.tensor_tensor(t0[:], lnr, sinv[:], op=ALU.mult)
            s.activation(dyv[:], t0[:], ACTF.Abs, scale=0.5)
            v.tensor_tensor(dxv[:], dxv[:], valid_r[:], op=ALU.mult)
            v.tensor_tensor(dyv[:], dyv[:], valid_r[:], op=ALU.mult)

            # per-j scalar bundle -> broadcast [P, 9M]:
            # cols: 0 bx1, 1 negbx1, 2 bw, 3 by1, 4 negby1, 5 bh, 6 ar4,
            #       7 bx2, 8 by2
            bsrc = sm.tile([1, 9 * M], dt.float32, name="bsrc", tag="bsrc")
            v.tensor_tensor(bsrc[:, 0 * M:1 * M], cxr, dxv[:], op=ALU.subtract)
            v.tensor_scalar(bsrc[:, 1 * M:2 * M], bsrc[:, 0 * M:1 * M], -1.0, None, op0=ALU.mult)
            v.tensor_scalar(bsrc[:, 2 * M:3 * M], dxv[:], 2.0, None, op0=ALU.mult)
            v.tensor_tensor(bsrc[:, 3 * M:4 * M], cyr, dyv[:], op=ALU.subtract)
            v.tensor_scalar(bsrc[:, 4 * M:5 * M], bsrc[:, 3 * M:4 * M], -1.0, None, op0=ALU.mult)
            v.tensor_scalar(bsrc[:, 5 * M:6 * M], dyv[:], 2.0, None, op0=ALU.mult)
            v.scalar_tensor_tensor(bsrc[:, 6 * M:7 * M], dxv[:], 4.0, dyv[:], op0=ALU.mult, op1=ALU.mult)
            v.tensor_tensor(bsrc[:, 7 * M:8 * M], cxr, dxv[:], op=ALU.add)
            v.tensor_tensor(bsrc[:, 8 * M:9 * M], cyr, dyv[:], op=ALU.add)
            BC_ps = pp.tile([P, 9 * M], dt.float32, name="BC_ps", tag="ps_s")
            te.matmul(BC_ps[:], onesb[:], bsrc[:], start=True, stop=True)
            BC = sm.tile([P, 9 * M], dt.float32, name="BC", tag="BC")
            s.copy(BC[:], BC_ps[:])
            col = lambda f, j: BC[:, f * M + j:f * M + j + 1]

            # select tables -> broadcast [P, 5M]: cx, cy, th, lnl, cls
            tsrc = sm.tile([1, 5 * M], dt.float32, name="tsrc", tag="tsrc")
            v.tensor_copy(tsrc[:, 0 * M:1 * M], cxr)
            v.tensor_copy(tsrc[:, 1 * M:2 * M], cyr)
            v.tensor_copy(tsrc[:, 2 * M:3 * M], thr)
            lnmx = wk("lnmx")
            v.tensor_scalar(lnmx[:], lnr, 1.0, None, op0=ALU.max)
            s.activation(tsrc[:, 3 * M:4 * M], lnmx[:], ACTF.Ln)
            v.tensor_copy(tsrc[:, 4 * M:5 * M], clsr)

            # ---------- IoU loop ----------
            rowpk = pl.tile([P, G], dt.float32, name="rowpk", tag="rowpk")
            v.memset(rowpk[:], 0.0)
            colpk = pl.tile([P, M], dt.float32, name="colpk", tag="colpk")

            for j in range(M):
                iwc = tp.tile([P, G], dt.float32, name="t_iwc", tag="tB")
                v._custom_dve(IWC_OP, out=iwc[:], in0=x2, in1=x1,
                              s0=col(7, j), s1=col(0, j))
                ihc = tp.tile([P, G], dt.float32, name="t_ihc", tag="tC")
                v._custom_dve(IWC_OP, out=ihc[:], in0=y2, in1=y1,
                              s0=col(8, j), s1=col(3, j))
                inter = tp.tile([P, G], dt.float32, name="t_inter", tag="tD")
                v.tensor_tensor(inter[:], iwc[:], ihc[:], op=ALU.mult)
                # r = 1/(aa + box_area_j); S >= 256 always so no eps clamp.
                # approx_fast (51 ULP) is ~5x cheaper than the exact divide.
                S = tp.tile([P, G], dt.float32, name="t_S", tag="tE")
                s.activation(S[:], aa[:], ACTF.Relu, bias=col(6, j))
                r = tp.tile([P, G], dt.float32, name="t_r", tag="tF")
                v.reciprocal_approx_fast(r[:], S[:])
                u = tp.tile([P, G], dt.float32, name="t_u", tag="tG")
                v.tensor_tensor(u[:], inter[:], r[:], op=ALU.mult)
                ub = u[:].bitcast(dt.uint32)
                jpk = tp.tile([P, G], dt.uint32, name="t_jpk", tag="tI2")
                ts_bits(jpk[:], ub, 0xFFFFFFE0, op0=ALU.bitwise_and,
                        s2=(31 - j) | BIT30, op1=ALU.bitwise_or)
                gpk = tp.tile([P, G], dt.uint32, name="t_gpk", tag="tH")
                stt_bits(gpk[:], jpk[:], 0xFFFFFC00, gcode[:],
                         op0=ALU.bitwise_and, op1=ALU.bitwise_or)
                v.tensor_reduce(colpk[:, j:j + 1], gpk[:].bitcast(dt.float32), axis=AX.X, op=ALU.max)
                v.tensor_tensor(rowpk[:], rowpk[:], jpk[:].bitcast(dt.float32), op=ALU.max)

            # ---------- per-anchor decode ----------
            umaxq = pl.tile([P, G], dt.float32, name="umaxq", tag="umaxq")
            ts_bits(umaxq[:].bitcast(dt.uint32), rowpk[:].bitcast(dt.uint32), 0xBFFFFFE0, op0=ALU.bitwise_and)
            jstar = pl.tile([P, G], dt.float32, name="jstar", tag="jstar")
            wst = tp.tile([P, G], dt.uint32, name="t_wst", tag="tH")
            ts_bits(wst[:], rowpk[:].bitcast(dt.uint32), 0x1F, op0=ALU.bitwise_and)
            v.tensor_copy(jstar[:], wst[:])
            v.tensor_scalar(jstar[:], jstar[:], -1.0, 31.0, op0=ALU.mult, op1=ALU.add)
            ge13 = pl.tile([P, G], dt.float32, name="ge13", tag="ge13")
            v.tensor_scalar(ge13[:], umaxq[:], T13, None, op0=ALU.is_ge)
            ge27 = pl.tile([P, G], dt.float32, name="ge27", tag="ge27")
            v.tensor_scalar(ge27[:], umaxq[:], T27, None, op0=ALU.is_ge)

            # ---------- column stats ----------
            cpT_ps = pp.tile([M, P], dt.float32, name="cpT", tag="ps_s")
            te.transpose(cpT_ps[:], colpk[:], ident[:])
            cpT = sm.tile([M, P], dt.float32, name="cpTs", tag="cpTs")
            s.copy(cpT[:], cpT_ps[:])
            mx8 = sm.tile([M, 8], dt.float32, name="mx8", tag="mx8")
            v.max(mx8[:], cpT[:])
            mi8 = sm.tile([M, 8], dt.uint32, name="mi8", tag="mi8")
            v.max_index(mi8[:], mx8[:], cpT[:])

            bun = sm.tile([M, 4], dt.float32, name="bun", tag="bun")
            v.tensor_copy(bun[:, 0:1], mi8[:, 0:1])                  # pstar
            pkb = mx8[:, 0:1].bitcast(dt.uint32)
            g10u = st.tile([M, 1], dt.uint32, name="g10u", tag="g10u")
            ts_bits(g10u[:], pkb, 0x3FF, op0=ALU.bitwise_and)
            v.tensor_copy(bun[:, 1:2], g10u[:])
            v.tensor_scalar(bun[:, 1:2], bun[:, 1:2], -1.0, 1023.0, op0=ALU.mult, op1=ALU.add)  # gstar
            ts_bits(bun[:, 2:3].bitcast(dt.uint32), pkb, 0xBFFFFC00, op0=ALU.bitwise_and)
            acol = st.tile([M, 1], dt.float32, name="acol", tag="acol")
            v.scalar_tensor_tensor(acol[:], bun[:, 0:1], 800.0, bun[:, 1:2], op0=ALU.mult, op1=ALU.add)
            docol = st.tile([M, 1], dt.float32, name="docol", tag="docol")
            v.tensor_scalar(docol[:], bun[:, 2:3], T13, None, op0=ALU.is_lt)
            validc_ps = pp.tile([M, 1], dt.float32, name="validc", tag="ps_s")
            te.transpose(validc_ps[:], valid_r[:], ident[0:1, 0:1])
            validc = st.tile([M, 1], dt.float32, name="validc_sb", tag="validc_sb")
            s.copy(validc[:], validc_ps[:])
            v.tensor_tensor(docol[:], docol[:], validc[:], op=ALU.mult)
            v.tensor_copy(bun[:, 3:4], docol[:])

            # vscat (column form): do * (j+1) * not-killed, dedup last-wins
            # kill_k = sum_l>k (a_l == a_k) * do_l >= 1, via PE ones-reduction
            arow_ps = pp.tile([1, M], dt.float32, name="arow_ps", tag="ps_s")
            te.transpose(arow_ps[:], acol[:], ident[:M, :M])
            arow = st.tile([1, M], dt.float32, name="arow", tag="arow")
            s.copy(arow[:], arow_ps[:])
            abc_ps = pp.tile([M, M], dt.float32, name="abc_ps", tag="ps_s")
            te.matmul(abc_ps[:], onesb[:, :M], arow[:], start=True, stop=True)
            eqm = sm.tile([M, M], dt.float32, name="eqm", tag="eqm")
            v.tensor_tensor(eqm[:], abc_ps[:], acol[:].broadcast_to((M, M)), op=ALU.is_equal)
            v.tensor_tensor(eqm[:], eqm[:], docol[:].broadcast_to((M, M)), op=ALU.mult)
            v.tensor_tensor(eqm[:], eqm[:], ltm[:], op=ALU.mult)
            killc_ps = pp.tile([M, 1], dt.float32, name="killc_ps", tag="ps_s")
            te.matmul(killc_ps[:], eqm[:], onesc[:M, :], start=True, stop=True)
            vscat_c = st.tile([M, 1], dt.float32, name="vscat_c", tag="vscat_c")
            v.tensor_scalar(vscat_c[:], killc_ps[:], 1.0, None, op0=ALU.is_lt)
            v.tensor_tensor(vscat_c[:], vscat_c[:], docol[:], op=ALU.mult)
            v.tensor_tensor(vscat_c[:], vscat_c[:], jp1c[:], op=ALU.mult)

            # override plane via rank-32 PE outer product:
            # ovc[p,g] = sum_j vscat_j * (p==pstar_j) * (g==gstar_j)
            Lm = sm.tile([M, P], dt.float32, name="Lm", tag="Lm")
            v.tensor_tensor(Lm[:], pio128[:], bun[:, 0:1].broadcast_to((M, P)), op=ALU.is_equal)
            v.tensor_tensor(Lm[:], Lm[:], vscat_c[:].broadcast_to((M, P)), op=ALU.mult)
            Rm = sm.tile([M, G], dt.float32, name="Rm", tag="Rm")
            v.tensor_tensor(Rm[:], gio800[:], bun[:, 1:2].broadcast_to((M, G)), op=ALU.is_equal)
            ovc = tp.tile([P, G], dt.float32, name="t_ovc", tag="tB")
            ovc_ps = pp.tile([P, 512], dt.float32, name="ovc_ps", tag="ovc_ps", bufs=1)
            te.matmul(ovc_ps[:, 0:512], Lm[:], Rm[:, 0:512], start=True, stop=True)
            s.copy(ovc[:, 0:512], ovc_ps[:, 0:512])
            te.matmul(ovc_ps[:, 0:G - 512], Lm[:], Rm[:, 512:G], start=True, stop=True)
            s.copy(ovc[:, 512:G], ovc_ps[:, 0:G - 512])
            ovf = pl.tile([P, G], dt.float32, name="ovf", tag="ovf")
            v.tensor_scalar(ovf[:], ovc[:], 0.0, None, op0=ALU.is_gt)

            jeff = pl.tile([P, G], dt.float32, name="jeff", tag="jeff")
            v.tensor_copy(jeff[:], jstar[:])
            ovj = tp.tile([P, G], dt.float32, name="t_ovj", tag="tC")
            v.tensor_scalar(ovj[:], ovc[:], 1.0, None, op0=ALU.subtract)
            ovf8 = tp.tile([P, G], dt.uint8, name="t_ovf8", tag="tD")
            v.tensor_copy(ovf8[:], ovf[:])
            v.copy_predicated(jeff[:], ovf8[:], ovj[:])

            pos = pl.tile([P, G], dt.float32, name="pos", tag="pos")
            v.tensor_tensor(pos[:], ge13[:], ovf[:], op=ALU.max)
            v.tensor_tensor(pos[:], pos[:], vmask[:], op=ALU.mult)
            w0 = pl.tile([P, G], dt.float32, name="w0", tag="w0")
            v.tensor_tensor(w0[:], ge27[:], ge13[:], op=ALU.subtract)
            nov = tp.tile([P, G], dt.float32, name="t_nov", tag="tD")
            v.tensor_scalar(nov[:], ovf[:], -1.0, 1.0, op0=ALU.mult, op1=ALU.add)
            v.tensor_tensor(w0[:], w0[:], nov[:], op=ALU.mult)
            v.tensor_scalar(w0[:], w0[:], -1.0, 1.0, op0=ALU.mult, op1=ALU.add)
            v.tensor_tensor(w0[:], w0[:], vmask[:], op=ALU.mult)

            # ---------- stream A: sum(w0 * c^2 * ln(1-c)) over [A,C] ----------
            # runs in two 5-chunk halves; each half stashes pure bf16 c^2 in
            # sq_half for the csel gather (which needs kstar, computed later).
            w0b16 = pl.tile([P, G], dt.bfloat16, name="w0b16", tag="w0b16")
            v.tensor_copy(w0b16[:], w0[:])
            sq_half = pl.tile([P, (NCHUNK // 2) * CHF], dt.bfloat16, name="sq_half", tag="sq_half")
            dsumall = sm.tile([P, NCHUNK], dt.float32, name="dsumall", tag="dsumall")
            clsv = cls_d.rearrange("(p g) c -> p (g c)", p=P)

            def stream_a(half):
                for hc in range(NCHUNK // 2):
                    ci = (NCHUNK // 2) * half + hc
                    cr = chp.tile([P, CHF], dt.float32, name="cr", tag="cr")
                    nc.sync.dma_start(cr[:, :], clsv[:, ci * CHF:(ci + 1) * CHF])
                    sq = sq_half[:, hc * CHF:(hc + 1) * CHF]
                    s.activation(sq, cr[:], ACTF.Square)
                    lg = chp.tile([P, CHF], dt.bfloat16, name="lg", tag="lg")
                    s.activation(lg[:], cr[:], ACTF.Ln, bias=1.0, scale=-1.0)
                    v.tensor_tensor(lg[:], lg[:], sq, op=ALU.mult)
                    w0bb = w0b16[:, ci * GC:(ci + 1) * GC].unsqueeze(-1).broadcast_to((P, GC, C))
                    v.scalar_tensor_tensor(lg[:], lg[:], 1.0, w0bb, op0=ALU.mult,
                                           op1=ALU.mult, accum_out=dsumall[:, ci:ci + 1])

            # stream A half 0 kicks off now: its DMA/scalar/DVE work fills
            # the gather phase's idle engine time (csel needs kstar later).
            stream_a(0)

            # ---------- field gather via PE one-hot matmul ----------
            # anchor (p0,g0), p0=4a+q: group q, chunk c=2a+h (h=g0//400), col
            # i=g0%400. OH[32q+j, i] = (jeff[4a+q, 400h+i] == j); gathered
            # field f lands at out row 5q+f via block-diag stationary tbl4.
            fieldpl = pl.tile([P, 5 * G], dt.float32, name="fieldpl", tag="fieldpl")
            tsrc5 = sm.tile([5, M], dt.float32, name="tsrc5", tag="tsrc5")
            for f in range(5):
                nc.sync.dma_start(tsrc5[f:f + 1, :], tsrc[:, f * M:(f + 1) * M])
            tpose_ps = pp.tile([M, 5], dt.float32, name="tpose_ps", tag="ps_s")
            te.transpose(tpose_ps[:], tsrc5[:], ident[0:5, 0:5])
            tblM = sm.tile([M, 5], dt.float32, name="tblM", tag="tblM")
            s.copy(tblM[:], tpose_ps[:])
            tbl4 = sm.tile([P, 20], dt.float32, name="tbl4", tag="tbl4")
            v.memset(tbl4[:], 0.0)
            for q in range(4):
                nc.sync.dma_start(tbl4[32 * q:32 * q + 32, 5 * q:5 * q + 5], tblM[:, :])
            CH2 = 400
            qselb = sm.tile([4, P], dt.bfloat16, name="qselb", tag="qselb")
            v.tensor_copy(qselb[:], qsel[:])
            jeffb = pl.tile([P, G], dt.bfloat16, name="jeffb", tag="jeffb")
            v.tensor_copy(jeffb[:], jeff[:])
            gps = None
            jstg = None
            for c in range(64):
                a_, h_ = c // 2, c % 2
                if h_ == 0:
                    jstg = chp.tile([4, G], dt.bfloat16, name="jstg", tag="jstg")
                    nc.sync.dma_start(jstg[:, :], jeffb[4 * a_:4 * a_ + 4, :])
                jrep_ps = pp.tile([P, CH2], dt.float32, name="jrep_ps", tag="jrep_ps", bufs=3)
                te.matmul(jrep_ps[:], qselb[:], jstg[:, CH2 * h_:CH2 * h_ + CH2], start=True, stop=True)
                oh = chp.tile([P, CH2], dt.float32, name="oh", tag="oh")
                v.tensor_scalar(oh[:], jrep_ps[:], iota32[:, 0:1], None, op0=ALU.is_equal)
                s3 = c % 3
                if s3 == 0:
                    gps = pp.tile([84, CH2], dt.float32, name="gps", tag="gps")
                te.matmul(gps[32 * s3:32 * s3 + 20, :], tbl4[:], oh[:], start=True, stop=True)
                if s3 == 2 or c == 63:
                    gsb = chp.tile([84, CH2], dt.float32, name="gsb", tag="gsb")
                    s.copy(gsb[:, :], gps[:, :])
                    for cc in range(c - s3, c + 1):
                        ca, chh = cc // 2, cc % 2
                        ss = cc % 3
                        for f in range(5):
                            nc.sync.dma_start(
                                fieldpl[4 * ca:4 * ca + 4, f * G + CH2 * chh:f * G + CH2 * chh + CH2],
                                gsb[32 * ss + f:32 * ss + 16 + f:5, :])
            cxg = fieldpl[:, 0 * G:1 * G]
            cyg = fieldpl[:, 1 * G:2 * G]
            thg = fieldpl[:, 2 * G:3 * G]
            lnlg = fieldpl[:, 3 * G:4 * G]
            clsg = fieldpl[:, 4 * G:5 * G]

            inR = tp.tile([P, G], dt.float32, name="t_inr", tag="tB")
            v.tensor_scalar(inR[:], clsg, 0.0, None, op0=ALU.is_ge)
            inR2 = tp.tile([P, G], dt.float32, name="t_inr2", tag="tC")
            v.tensor_scalar(inR2[:], clsg, float(C - 1), None, op0=ALU.is_le)
            v.tensor_tensor(inR[:], inR[:], inR2[:], op=ALU.mult)
            v.tensor_tensor(pos[:], pos[:], inR[:], op=ALU.mult)
            kstar = pl.tile([P, G], dt.float32, name="kstar", tag="kstar")
            v.tensor_scalar(kstar[:], clsg, float(C - 1), 0.0, op0=ALU.min, op1=ALU.max)

            if debug:
                nc.sync.dma_start(dbg["dbg_umaxq"][:, :], umaxq[:])
                nc.sync.dma_start(dbg["dbg_w0"][:, :], w0[:])
                nc.sync.dma_start(dbg["dbg_pos"][:, :], pos[:])
                nc.sync.dma_start(dbg["dbg_jeff"][:, :], jeff[:])
                nc.sync.dma_start(dbg["dbg_colpk"][:, :], colpk[:].bitcast(dt.uint32))
                nc.sync.dma_start(dbg["dbg_rowpk"][:, :], rowpk[:].bitcast(dt.uint32))
                nc.sync.dma_start(dbg["dbg_ovc"][:, :], ovc[:])
                nc.sync.dma_start(dbg["dbg_clsg"][:, :], clsg)

            # k-masks for the csel select (40 planes would be too much SBUF;
            # compute one [P,G] mask per k on the fly inside the chunk loop
            # would redo work 10x; instead compute all 40 as uint8? -> keep
            # it simple: compute mask per (k) once into a temp and apply to
            # all 10 chunk slices immediately (csel is chunk-agnostic: data
            # comes from the chunk tile, mask from the full plane slice).

            # csel gather from the stashed bf16 c^2 halves (stream A below):
            # csq[p,g] = sq_half[p, g, kstar]; csel = sqrt(csq), clamped.
            csq = pl.tile([P, G], dt.bfloat16, name="csq", tag="csq")
            for half in range(2):
                if half == 1:
                    stream_a(1)
                gsl = slice(half * (G // 2), (half + 1) * (G // 2))
                for k in range(C):
                    mk_ = st.tile([P, G // 2], dt.uint8, name="t_mk", tag="t_mk")
                    v.tensor_scalar(mk_[:], kstar[:, gsl], float(k), None, op0=ALU.is_equal)
                    v.copy_predicated(
                        csq[:, gsl], mk_[:],
                        sq_half[:].rearrange("p (g c) -> p g c", c=C)[:, :, k])
            dsum = sm.tile([P, 1], dt.float32, name="dsum", tag="dsum")
            v.tensor_reduce(dsum[:], dsumall[:], axis=AX.X, op=ALU.add)

            # csel = sqrt(csq) clamped to [LO, HI] (bf16 c^2 -> ~0.2% on the
            # small per-anchor correction terms only)
            cselb = pl.tile([P, G], dt.float32, name="cselb", tag="cselb")
            v.tensor_scalar(cselb[:], csq[:], float(LO * LO), float(HI * HI),
                            op0=ALU.max, op1=ALU.min)
            csel = cselb
            s.activation(csel[:], csel[:], ACTF.Sqrt)
            if debug:
                nc.sync.dma_start(dbg["dbg_csel"][:, :], csel[:])

            # ---------- delta terms ----------
            acc = sm.tile([P, 4], dt.float32, name="acc", tag="acc")
            lnc = tp.tile([P, G], dt.float32, name="t_lnc", tag="tB")
            s.activation(lnc[:], csel[:], ACTF.Ln)
            ln1c = tp.tile([P, G], dt.float32, name="t_ln1c", tag="tC")
            s.activation(ln1c[:], csel[:], ACTF.Ln, bias=1.0, scale=-1.0)
            om2 = tp.tile([P, G], dt.float32, name="t_om2", tag="tD")
            v.tensor_scalar(om2[:], csel[:], -1.0, 1.0, op0=ALU.mult, op1=ALU.add)
            v.tensor_tensor(om2[:], om2[:], om2[:], op=ALU.mult)
            c2 = tp.tile([P, G], dt.float32, name="t_c2", tag="tE")
            v.tensor_tensor(c2[:], csel[:], csel[:], op=ALU.mult)
            v.tensor_tensor(om2[:], om2[:], lnc[:], op=ALU.mult)
            v.scalar_tensor_tensor(om2[:], om2[:], 1.0, pos[:], op0=ALU.mult, op1=ALU.mult, accum_out=acc[:, 0:1])
            v.tensor_tensor(c2[:], c2[:], ln1c[:], op=ALU.mult)
            v.scalar_tensor_tensor(c2[:], c2[:], 1.0, pos[:], op0=ALU.mult, op1=ALU.mult, accum_out=acc[:, 1:2])
            npt = tp.tile([P, G], dt.float32, name="t_npt", tag="tF")
            v.tensor_scalar(npt[:], pos[:], 0.0, 0.0, op0=ALU.add, op1=ALU.add, accum_out=acc[:, 2:3])

            # ---------- regression ----------
            regr = chp.tile([P, 4 * G], dt.float32, name="cr", tag="cr")
            nc.sync.dma_start(regr[:, :], reg_d.rearrange("(p g) c -> p (g c)", p=P))
            reg0 = regr[:, 0:4 * G:4]
            reg1 = regr[:, 1:4 * G:4]
            reg2 = regr[:, 2:4 * G:4]
            reg3 = regr[:, 3:4 * G:4]

            rw2 = tp.tile([P, G], dt.float32, name="t_rw2", tag="tA")
            v.reciprocal_approx_fast(rw2[:], aw[:])
            rh2 = tp.tile([P, G], dt.float32, name="t_rh2", tag="tB")
            v.reciprocal_approx_fast(rh2[:], ah[:])
            lnal = tp.tile([P, G], dt.float32, name="t_lnal", tag="tC")
            aw2 = tp.tile([P, G], dt.float32, name="t_aw2", tag="tD")
            v.tensor_tensor(aw2[:], aw[:], aw[:], op=ALU.mult)
            ah2 = tp.tile([P, G], dt.float32, name="t_ah2", tag="tE")
            v.tensor_tensor(ah2[:], ah[:], ah[:], op=ALU.mult)
            v.tensor_tensor(aw2[:], aw2[:], ah2[:], op=ALU.add)
            s.activation(lnal[:], aw2[:], ACTF.Ln)

            rsum = pl.tile([P, G], dt.float32, name="rsum", tag="rsum")
            dtl = tp.tile([P, G], dt.float32, name="t_dtl", tag="tF")
            dd = tp.tile([P, G], dt.float32, name="t_dd", tag="tG")

            def sl1_accum(first):
                m_ = tp.tile([P, G], dt.float32, name="t_sl1m", tag="tD")
                v.tensor_scalar(m_[:], dd[:], 1.0, None, op0=ALU.min)
                v.tensor_tensor(m_[:], m_[:], m_[:], op=ALU.mult)
                rl_ = tp.tile([P, G], dt.float32, name="t_sl1r", tag="tE")
                s.activation(rl_[:], dd[:], ACTF.Relu, bias=biasc[:, 1:2])
                if first:
                    v.scalar_tensor_tensor(rsum[:], m_[:], 0.5, rl_[:], op0=ALU.mult, op1=ALU.add)
                else:
                    v.scalar_tensor_tensor(m_[:], m_[:], 0.5, rl_[:], op0=ALU.mult, op1=ALU.add)
                    v.tensor_tensor(rsum[:], rsum[:], m_[:], op=ALU.add)

            # d0: |(cxg - (x1+x2)/2) * 2/aw - reg0|
            v.tensor_tensor(dtl[:], x1, x2, op=ALU.add)
            v.tensor_scalar(dtl[:], dtl[:], 0.5, None, op0=ALU.mult)
            v.tensor_tensor(dtl[:], cxg, dtl[:], op=ALU.subtract)
            v.tensor_tensor(dtl[:], dtl[:], rw2[:], op=ALU.mult)
            v.tensor_scalar(dtl[:], dtl[:], 2.0, None, op0=ALU.mult)
            v.tensor_tensor(dtl[:], dtl[:], reg0, op=ALU.subtract)
            s.activation(dd[:], dtl[:], ACTF.Abs)
            sl1_accum(True)
            # d1
            v.tensor_tensor(dtl[:], y1, y2, op=ALU.add)
            v.tensor_scalar(dtl[:], dtl[:], 0.5, None, op0=ALU.mult)
            v.tensor_tensor(dtl[:], cyg, dtl[:], op=ALU.subtract)
            v.tensor_tensor(dtl[:], dtl[:], rh2[:], op=ALU.mult)
            v.tensor_scalar(dtl[:], dtl[:], 2.0, None, op0=ALU.mult)
            v.tensor_tensor(dtl[:], dtl[:], reg1, op=ALU.subtract)
            s.activation(dd[:], dtl[:], ACTF.Abs)
            sl1_accum(False)
            # d2: |sin(thg - reg2)| with range reduction into (-pi, pi]
            v.tensor_tensor(dtl[:], thg, reg2, op=ALU.subtract)
            TWO_PI = float(f32(2.0 * math.pi))
            PI_ = float(f32(math.pi))
            gtpi = tp.tile([P, G], dt.float32, name="gtpi", tag="tA")
            for _ in range(2):
                v.tensor_scalar(gtpi[:], dtl[:], PI_, None, op0=ALU.is_gt)
                v.scalar_tensor_tensor(dtl[:], gtpi[:], -TWO_PI, dtl[:], op0=ALU.mult, op1=ALU.add)
            v.tensor_scalar(gtpi[:], dtl[:], -PI_, None, op0=ALU.is_lt)
            v.scalar_tensor_tensor(dtl[:], gtpi[:], TWO_PI, dtl[:], op0=ALU.mult, op1=ALU.add)
            s.activation(dtl[:], dtl[:], ACTF.Sin)
            s.activation(dd[:], dtl[:], ACTF.Abs)
            sl1_accum(False)
            # d3: |2*(lnlg - 0.5*ln(aw^2+ah^2)) - reg3|
            v.scalar_tensor_tensor(dtl[:], lnal[:], 0.5, lnlg, op0=ALU.mult, op1=ALU.subtract)
            v.tensor_scalar(dtl[:], dtl[:], -2.0, None, op0=ALU.mult)
            v.tensor_tensor(dtl[:], dtl[:], reg3, op=ALU.subtract)
            s.activation(dd[:], dtl[:], ACTF.Abs)
            sl1_accum(False)

            if debug:
                nc.sync.dma_start(dbg["dbg_rsum"][:, :], rsum[:])
            v.scalar_tensor_tensor(rsum[:], rsum[:], 1.0, pos[:], op0=ALU.mult, op1=ALU.mult,
                                   accum_out=acc[:, 3:4])

            # ---------- final reduction (PE ones-matmul over partitions) ----------
            accr_ps = pp.tile([1, 4], dt.float32, name="accr_ps", tag="ps_s")
            te.matmul(accr_ps[:], onesc[:], acc[:], start=True, stop=True)
            dsr_ps = pp.tile([1, 1], dt.float32, name="dsr_ps", tag="ps_s")
            te.matmul(dsr_ps[:], onesc[:], dsum[:], start=True, stop=True)
            outsb = sm.tile([1, N_OUT], dt.float32, name="outsb", tag="outsb")
            v.memset(outsb[:], 0.0)
            v.tensor_copy(outsb[:, 0:1], dsr_ps[:])
            v.tensor_copy(outsb[:, 1:5], accr_ps[:])
            nc.sync.dma_start(out_d[None, :], outsb[:])
    nc.finalize()
    return nc


_CACHED = {}


def _get_nc(debug=False):
    key = bool(debug)
    if key not in _CACHED:
        _CACHED[key] = build_bass(debug=key)
    return _CACHED[key]


def assemble(outs):
    cls_l, reg_l = [], []
    for o in outs:
        o0, o1, o2, o3, o4 = (f32(o[i]) for i in range(5))
        np1 = max(o3, f32(1.0))
        cls_l.append((-(f32(1.0) - ALPHA) * (o0 - o2) - ALPHA * o1) / np1)
        reg_l.append(REG_W * o4 / np1)
    return f32(np.mean(np.array(cls_l, dtype=f32)) + np.mean(np.array(reg_l, dtype=f32)))


def make_in_maps(classifications, regressions, anchors_pos, annotations):
    consts = host_constants()
    anc_pad = np.empty((P * G, 4), dtype=f32)
    anc_pad[:A] = anchors_pos
    anc_pad[A:, 0] = anc_pad[A:, 1] = -2.0e6
    anc_pad[A:, 2] = anc_pad[A:, 3] = -1.0e6
    in_maps = []
    for b in range(classifications.shape[0]):
        cls_pad = np.full((P * G, C), 0.5, dtype=f32)
        cls_pad[:A] = classifications[b]
        reg_pad = np.zeros((P * G, 4), dtype=f32)
        reg_pad[:A] = regressions[b]
        m = {
            "classification": cls_pad,
            "regression": reg_pad,
            "anchors": anc_pad,
            "annotation": np.ascontiguousarray(annotations[b], dtype=np.float32),
        }
        m.update(consts)
        in_maps.append(m)
    return in_maps


def kernel(classifications, regressions, anchors_pos, annotations):
    from concourse.bass_utils import run_bass_kernel_spmd
    nc = _get_nc(debug=False)
    in_maps = make_in_maps(classifications, regressions, anchors_pos, annotations)
    res = run_bass_kernel_spmd(nc, in_maps, list(range(classifications.shape[0])))
    outs = [res.results[b]["out"] for b in range(classifications.shape[0])]
    return np.array(assemble(outs), dtype=np.float32)

